# revision 1
# baseline (speedup 1.0000x reference)
"""GQA kernel for Trainium2, sharded over 8 NeuronCores.

Problem: B=2, S=2048, H=2048, NH=16 q-heads, KVH=4 kv-heads, D=128.
Sharding: core c -> (batch b = c//4, kv-head k = c%4). Each core computes the
full attention for its 4 query heads + its kv head on its batch, plus the
row-parallel partial of the output projection. Host sums the 4 partials per
batch.

Layout strategy (everything transposed so nothing large ever needs an
on-chip transpose):
  - hiddenT [H, S] (host pre-transposed), bf16
  - qT/kT/vT [D, S] from projection matmuls (moving dim = tokens)
  - scoresT [kv, q] = kT_block.T @ qT  (single K=128 matmul per block)
  - softmax: exp on ACT (scale=1/sqrt(D) fused), no max subtraction (scores
    are O(5), exp is safe in fp32/bf16), causal masks as 0/1 bf16 multiplies
    on diagonal blocks only; fully-masked blocks skipped.
  - denom: DVE accumulates exp tiles, one ones-matmul reduces partitions.
  - attn_outT [D, q] = v_block(lhsT) x expT(rhs) accumulated over kv tiles;
    this IS xT for the o-projection (no transpose).
  - o [tok, H] partial = xT(lhsT) x WoT(rhs); bias via K=1 aug matmul on
    kvh==0 cores only; DMA'd straight from PSUM to DRAM.
"""

import numpy as np
import ml_dtypes

import concourse.bass as bass
import concourse.mybir as mybir
import concourse.tile as tile
from concourse import bacc

BF16 = ml_dtypes.bfloat16
F32 = mybir.dt.float32
BF = mybir.dt.bfloat16

B, S, H = 2, 2048, 2048
NH, KVH, D = 16, 4, 128
G = NH // KVH  # q heads per kv head / per core
N_CORES = 8
SCALE = 1.0 / float(np.sqrt(D))

SQ = 512              # q-chunk (psum free width)
NQC = S // SQ         # 4 q chunks
NKT = S // 128        # 16 kv tiles / token tiles
NHT = H // 128        # 16 hidden k-tiles
ROWS = G + 2          # 6 projection row-blocks: 4 q heads, k, v


def build_nc(num_devices: int = N_CORES) -> bass.Bass:
    nc = bacc.Bacc("TRN2", num_devices=num_devices)

    hT = nc.dram_tensor("hT", [H, S], BF, kind="ExternalInput").ap()
    wqkvT = nc.dram_tensor("wqkvT", [H, ROWS * 128], BF, kind="ExternalInput").ap()
    bqkv = nc.dram_tensor("bqkv", [128, ROWS], F32, kind="ExternalInput").ap()
    cosT = nc.dram_tensor("cosT", [128, S], F32, kind="ExternalInput").ap()
    sinTs = nc.dram_tensor("sinTs", [128, S], F32, kind="ExternalInput").ap()
    rotT = nc.dram_tensor("rotT", [128, 128], F32, kind="ExternalInput").ap()
    masks = nc.dram_tensor("masks", [128, 4 * SQ], BF, kind="ExternalInput").ap()
    woT = nc.dram_tensor("woT", [G * 128, H], BF, kind="ExternalInput").ap()
    wob = nc.dram_tensor("wob", [1, H], BF, kind="ExternalInput").ap()
    id128 = nc.dram_tensor("id128", [128, 128], BF, kind="ExternalInput").ap()
    out = nc.dram_tensor("out", [S, H], F32, kind="ExternalOutput").ap()

    with tile.TileContext(nc) as tc:
        with (
            tc.tile_pool(name="consts", bufs=1) as consts,
            tc.tile_pool(name="persist", bufs=1) as persist,
            tc.tile_pool(name="w3", bufs=3) as w3,
            tc.tile_pool(name="w2", bufs=2) as w2,
            tc.tile_pool(name="ps2", bufs=2, space="PSUM") as ps2,
            tc.tile_pool(name="ps3", bufs=3, space="PSUM") as ps3,
        ):
            # ---- constants ----
            cos_sb = consts.tile([128, S], F32, tag="cos", name="cos")
            nc.sync.dma_start(out=cos_sb, in_=cosT)
            sin_sb = consts.tile([128, S], F32, tag="sin", name="sin")
            nc.sync.dma_start(out=sin_sb, in_=sinTs)
            mask_sb = consts.tile([128, 4 * SQ], BF, tag="mask", name="mask")
            nc.sync.dma_start(out=mask_sb, in_=masks)
            bias_sb = consts.tile([128, ROWS], F32, tag="bias", name="bias")
            nc.sync.dma_start(out=bias_sb, in_=bqkv)
            id_sb = consts.tile([128, 128], BF, tag="id", name="id")
            nc.sync.dma_start(out=id_sb, in_=id128)
            rt_sb = consts.tile([128, 128], F32, tag="rt", name="rt")
            nc.sync.dma_start(out=rt_sb, in_=rotT)
            wob_sb = consts.tile([1, H], BF, tag="wob", name="wob")
            nc.sync.dma_start(out=wob_sb, in_=wob)
            ones_f = consts.tile([128, 1], F32, tag="ones_f", name="ones_f")
            nc.vector.memset(ones_f, 1.0)
            ones_r = consts.tile([1, 128], BF, tag="ones_r", name="ones_r")
            nc.vector.memset(ones_r, 1.0)
            ones_rf = consts.tile([1, 128], F32, tag="ones_rf", name="ones_rf")
            nc.vector.memset(ones_rf, 1.0)

            # ---- weight + activation loads ----
            wq_sb = []
            for kt in range(NHT):
                t = persist.tile([128, ROWS * 128], BF, tag=f"wq{kt}", name=f"wq{kt}")
                nc.sync.dma_start(out=t, in_=wqkvT[kt * 128:(kt + 1) * 128, :])
                wq_sb.append(t)
            wo_sb = []
            for g in range(G):
                t = persist.tile([128, H], BF, tag=f"wo{g}", name=f"wo{g}")
                nc.sync.dma_start(out=t, in_=woT[g * 128:(g + 1) * 128, :])
                wo_sb.append(t)

            # ---- phase 1: QKV projection + RoPE ----
            # row-blocks: 0..3 = q heads (RoPE), 4 = k (RoPE), 5 = v (plain)
            qk_sb = [persist.tile([128, S], BF, tag=f"qk{m}", name=f"qk{m}") for m in range(G + 1)]
            vT_sb = persist.tile([128, S], BF, tag="vT", name="vT")
            for c in range(NQC):
                cs = slice(c * SQ, (c + 1) * SQ)
                h_sb = []
                for kt in range(NHT):
                    t = w2.tile([128, SQ], BF, tag=f"h{kt}", name=f"h{kt}")
                    nc.sync.dma_start(
                        out=t, in_=hT[kt * 128:(kt + 1) * 128, cs]
                    )
                    h_sb.append(t)
                for m in range(ROWS):
                    ps = ps3.tile([128, SQ], F32, tag="mm", name="mm")
                    for kt in range(NHT):
                        nc.tensor.matmul(
                            ps,
                            wq_sb[kt][:, m * 128:(m + 1) * 128],
                            h_sb[kt],
                            start=(kt == 0),
                            stop=(kt == NHT - 1),
                        )
                    if m < ROWS - 1:
                        # q or k: bias then RoPE
                        tmp = w3.tile([128, SQ], F32, tag="rope_in", name="rope_in")
                        nc.scalar.activation(
                            tmp, ps, mybir.ActivationFunctionType.Identity,
                            bias=bias_sb[:, m:m + 1],
                        )
                        rp = ps2.tile([128, SQ], F32, tag="small", name="small")
                        nc.tensor.matmul(rp, rt_sb, tmp, start=True, stop=True)
                        rot = w3.tile([128, SQ], F32, tag="rope_rot", name="rope_rot")
                        nc.vector.tensor_mul(rot, rp, sin_sb[:, cs])
                        nc.vector.tensor_mul(tmp, tmp, cos_sb[:, cs])
                        nc.vector.tensor_add(qk_sb[m][:, cs], tmp, rot)
                    else:
                        nc.scalar.activation(
                            vT_sb[:, cs], ps,
                            mybir.ActivationFunctionType.Identity,
                            bias=bias_sb[:, m:m + 1],
                        )

            # transpose vT -> v blocks [kv,128 x D,128]
            v_sb = []
            for j in range(NKT):
                tp = ps2.tile([128, 128], BF, tag="small", name="small")
                nc.tensor.transpose(tp, vT_sb[:, j * 128:(j + 1) * 128], id_sb)
                vb = persist.tile([128, 128], BF, tag=f"v{j}", name=f"v{j}")
                nc.scalar.copy(vb, tp)
                v_sb.append(vb)

            kT = qk_sb[G]

            # ---- phase 2: attention ----
            xT_sb = [persist.tile([128, S], BF, tag=f"xT{h}", name=f"xT{h}") for h in range(G)]
            for h in range(G):
                for c in range(NQC):
                    cs = slice(c * SQ, (c + 1) * SQ)
                    njt = 4 * c + 4  # kv tiles 0 .. 4c+3
                    av = ps2.tile([128, SQ], F32, tag="acc", name="acc")
                    dacc = w2.tile([128, SQ], F32, tag="dacc", name="dacc")
                    for j in range(njt):
                        sc = ps3.tile([128, SQ], F32, tag="mm", name="mm")
                        nc.tensor.matmul(
                            sc,
                            kT[:, j * 128:(j + 1) * 128],
                            qk_sb[h][:, cs],
                            start=True, stop=True,
                        )
                        ex = w3.tile([128, SQ], BF, tag="exp", name="exp")
                        nc.scalar.activation(
                            ex, sc, mybir.ActivationFunctionType.Exp,
                            scale=SCALE,
                        )
                        diag = j - 4 * c  # >= 0 for the 4 diagonal tiles
                        if diag >= 0:
                            nc.vector.tensor_mul(
                                ex, ex, mask_sb[:, diag * SQ:(diag + 1) * SQ]
                            )
                        if j == 0:
                            nc.vector.tensor_copy(dacc, ex)
                        else:
                            nc.vector.tensor_add(dacc, dacc, ex)
                        nc.tensor.matmul(
                            av, v_sb[j], ex,
                            start=(j == 0), stop=(j == njt - 1),
                        )
                    dn = ps2.tile([1, SQ], F32, tag="small", name="small")
                    nc.tensor.matmul(dn, ones_f, dacc, start=True, stop=True)
                    rd = w2.tile([1, SQ], F32, tag="rd", name="rd")
                    nc.vector.reciprocal(rd, dn)
                    bc = ps2.tile([128, SQ], F32, tag="small", name="small")
                    nc.tensor.matmul(bc, ones_rf, rd, start=True, stop=True)
                    bcs = w2.tile([128, SQ], F32, tag="bcs", name="bcs")
                    nc.scalar.copy(bcs, bc)
                    nc.vector.tensor_mul(xT_sb[h][:, cs], av, bcs)

            # ---- phase 3: output projection (row-parallel partial) ----
            for t in range(NKT):
                ts_ = slice(t * 128, (t + 1) * 128)
                for n in range(NQC):
                    ns = slice(n * SQ, (n + 1) * SQ)
                    op = ps2.tile([128, SQ], F32, tag="acc", name="acc")
                    for g in range(G):
                        nc.tensor.matmul(
                            op, xT_sb[g][:, ts_], wo_sb[g][:, ns],
                            start=(g == 0), stop=False,
                        )
                    nc.tensor.matmul(
                        op, ones_r, wob_sb[:, ns], start=False, stop=True,
                    )
                    o_sb = w3.tile([128, SQ], F32, tag="o_sb", name="o_sb")
                    nc.scalar.copy(o_sb, op)
                    nc.sync.dma_start(out=out[ts_, ns], in_=o_sb)
    nc.compile()
    return nc


def make_in_maps(hidden_states, cos, sin, Wq, bq, Wk, bk, Wv, bv, Wo, bo):
    """Host-side shard/pack. Returns list of 8 input dicts."""
    f32 = np.float32
    cosT = np.ascontiguousarray(cos.T).astype(f32)
    sinTs = np.ascontiguousarray(sin.T).astype(f32)
    R = np.zeros((128, 128), f32)
    for d in range(64):
        R[d, d + 64] = -1.0
        R[d + 64, d] = 1.0
    rotT = np.ascontiguousarray(R.T)
    # causal 0/1 masks for the 4 diagonal offsets
    p = np.arange(128)[:, None]
    f = np.arange(SQ)[None, :]
    masks = np.concatenate(
        [(f >= (128 * i + p)) for i in range(4)], axis=1
    ).astype(BF16)
    id128 = np.eye(128, dtype=BF16)

    in_maps = []
    for core in range(N_CORES):
        b, k = core // 4, core % 4
        hT = np.ascontiguousarray(np.asarray(hidden_states[b]).T).astype(BF16)
        wq = Wq[512 * k:512 * (k + 1)]            # [512, H]
        wk = Wk[128 * k:128 * (k + 1)]            # [128, H]
        wv = Wv[128 * k:128 * (k + 1)]
        wqkvT = np.ascontiguousarray(
            np.concatenate([wq, wk, wv], axis=0).T
        ).astype(BF16)                             # [H, 768]
        bqkv = np.concatenate(
            [bq[512 * k:512 * (k + 1)], bk[128 * k:128 * (k + 1)],
             bv[128 * k:128 * (k + 1)]]
        ).astype(f32).reshape(ROWS, 128).T.copy()  # [128, ROWS]
        woT = np.ascontiguousarray(Wo[:, 512 * k:512 * (k + 1)].T).astype(BF16)
        wob = (bo if k == 0 else np.zeros_like(bo)).astype(BF16).reshape(1, H)
        in_maps.append({
            "hT": hT, "wqkvT": wqkvT, "bqkv": bqkv,
            "cosT": cosT, "sinTs": sinTs, "masks": masks, "rotT": rotT,
            "woT": woT, "wob": wob, "id128": id128,
        })
    return in_maps


_NC = None


def kernel(**inputs) -> np.ndarray:
    global _NC
    from concourse.bass_utils import run_bass_kernel_spmd

    if _NC is None:
        _NC = build_nc()
    in_maps = make_in_maps(**inputs)
    res = run_bass_kernel_spmd(_NC, in_maps, core_ids=list(range(N_CORES)))
    out = np.zeros((B, S, H), np.float32)
    for core in range(N_CORES):
        out[core // 4] += res.results[core]["out"]
    return out



# revision 2
# speedup vs baseline: 1.0616x; 1.0616x over previous
"""GQA kernel for Trainium2, sharded over 8 NeuronCores.

Problem: B=2, S=2048, H=2048, NH=16 q-heads, KVH=4 kv-heads, D=128.
Sharding: core c -> (batch b = c//4, kv-head k = c%4). Each core computes the
full attention for its 4 query heads + its kv head on its batch, plus the
row-parallel partial of the output projection. Host sums the 4 partials per
batch and adds the output bias.

v2 design (single fused pass per 512-token q-chunk):
  for c in 0..3:
    prefetch h(c+1); QKV projection + RoPE for chunk c (rows k,v,q0..q3);
    oproj for chunk c-1; transpose v tiles of chunk c;
    attention for chunk c in two head-pair sweeps:
      per kv tile j: 2 score matmuls (kT[j] stationary), ONE exp over the
      [128, 2, w] head-pair mega-tile (causally trimmed width w), triangular
      mask only on the 128-wide diagonal block, denominator accumulated in
      bf16 alternating DVE/GPSIMD, attn@V accumulated in PSUM;
      per head: ones-matmul partition-reduce -> reciprocal_approx_fast ->
      gpsimd partition_broadcast -> normalize into xT (bf16).
  oproj chunk 3.
Output partials written as bf16; host upcasts, sums, and adds bo.
"""

import numpy as np
import ml_dtypes

import concourse.bass as bass
import concourse.mybir as mybir
import concourse.tile as tile
from concourse import bacc

BF16 = ml_dtypes.bfloat16
F32 = mybir.dt.float32
BF = mybir.dt.bfloat16

B, S, H = 2, 2048, 2048
NH, KVH, D = 16, 4, 128
G = NH // KVH  # q heads per kv head / per core
N_CORES = 8
SCALE = 1.0 / float(np.sqrt(D))

SQ = 512              # q-chunk width
NQC = S // SQ         # 4 q chunks
NKT = S // 128        # 16 kv tiles / token tiles
NHT = H // 128        # 16 hidden k-tiles
ROWS = G + 2          # 6 projection row-blocks: 4 q heads, k, v
EXPF = mybir.ActivationFunctionType.Exp
IDF = mybir.ActivationFunctionType.Identity


def build_nc(num_devices: int = N_CORES) -> bass.Bass:
    nc = bacc.Bacc("TRN2", num_devices=num_devices)

    hT = nc.dram_tensor("hT", [H, S], BF, kind="ExternalInput").ap()
    wqkvT = nc.dram_tensor("wqkvT", [H, ROWS * 128], BF, kind="ExternalInput").ap()
    bqkv = nc.dram_tensor("bqkv", [128, ROWS], F32, kind="ExternalInput").ap()
    cosT = nc.dram_tensor("cosT", [128, S], BF, kind="ExternalInput").ap()
    sinT = nc.dram_tensor("sinT", [128, S], BF, kind="ExternalInput").ap()
    rotT = nc.dram_tensor("rotT", [128, 128], BF, kind="ExternalInput").ap()
    masks2 = nc.dram_tensor("masks2", [128, 256], BF, kind="ExternalInput").ap()
    woT = nc.dram_tensor("woT", [G * 128, H], BF, kind="ExternalInput").ap()
    id128 = nc.dram_tensor("id128", [128, 128], BF, kind="ExternalInput").ap()
    out = nc.dram_tensor("out", [S, H], BF, kind="ExternalOutput").ap()

    with tile.TileContext(nc) as tc:
        with (
            tc.tile_pool(name="consts", bufs=1) as consts,
            tc.tile_pool(name="persist", bufs=1) as persist,
            tc.tile_pool(name="hbuf", bufs=2) as hbuf,
            tc.tile_pool(name="work", bufs=3) as work,
            tc.tile_pool(name="work2", bufs=2) as work2,
            tc.tile_pool(name="psQ", bufs=2, space="PSUM") as psQ,
            tc.tile_pool(name="psS", bufs=2, space="PSUM") as psS,
            tc.tile_pool(name="psAV", bufs=2, space="PSUM") as psAV,
        ):
            # ---- small constants (first in DMA queue) ----
            bias_sb = consts.tile([128, ROWS], F32, tag="bias", name="bias")
            nc.sync.dma_start(out=bias_sb, in_=bqkv)
            rt_sb = consts.tile([128, 128], BF, tag="rt", name="rt")
            nc.sync.dma_start(out=rt_sb, in_=rotT)
            mask_sb = consts.tile([128, 2, 128], BF, tag="mask", name="mask")
            nc.sync.dma_start(out=mask_sb, in_=masks2)
            id_sb = consts.tile([128, 128], BF, tag="id", name="id")
            nc.sync.dma_start(out=id_sb, in_=id128)
            ones_f = consts.tile([128, 1], BF, tag="ones_f", name="ones_f")
            nc.vector.memset(ones_f, 1.0)
            # preload the exp activation table while DMA streams
            warm_in = consts.tile([128, 1], F32, tag="warm_in", name="warm_in")
            nc.vector.memset(warm_in, 0.0)
            warm_out = consts.tile([128, 1], BF, tag="warm_out", name="warm_out")
            nc.scalar.activation(warm_out, warm_in, EXPF)

            cos_sb = persist.tile([128, S], BF, tag="cos", name="cos")
            sin_sb = persist.tile([128, S], BF, tag="sin", name="sin")
            nc.sync.dma_start(out=cos_sb[:, 0:SQ], in_=cosT[:, 0:SQ])
            nc.sync.dma_start(out=sin_sb[:, 0:SQ], in_=sinT[:, 0:SQ])

            # ---- weights + chunk-0 hidden, interleaved by k-tile ----
            wq_sb = []
            h_tiles = [[None] * NHT for _ in range(NQC)]
            for kt in range(NHT):
                t = persist.tile([128, ROWS * 128], BF, tag=f"wq{kt}", name=f"wq{kt}")
                nc.sync.dma_start(out=t, in_=wqkvT[kt * 128:(kt + 1) * 128, :])
                wq_sb.append(t)
                ht = hbuf.tile([128, SQ], BF, tag=f"h{kt}", name=f"h0_{kt}")
                nc.sync.dma_start(out=ht, in_=hT[kt * 128:(kt + 1) * 128, 0:SQ])
                h_tiles[0][kt] = ht
            for c in range(1, NQC):
                cs = slice(c * SQ, (c + 1) * SQ)
                nc.sync.dma_start(out=cos_sb[:, cs], in_=cosT[:, cs])
                nc.sync.dma_start(out=sin_sb[:, cs], in_=sinT[:, cs])
            wo_sb = []
            for g in range(G):
                t = persist.tile([128, H], BF, tag=f"wo{g}", name=f"wo{g}")
                nc.sync.dma_start(out=t, in_=woT[g * 128:(g + 1) * 128, :])
                wo_sb.append(t)

            # persistent activations (bf16)
            qk_sb = [persist.tile([128, S], BF, tag=f"qk{m}", name=f"qk{m}")
                     for m in range(G + 1)]  # 0..3 q heads, 4 = k
            vT_sb = persist.tile([128, S], BF, tag="vT", name="vT")
            v_sb = [persist.tile([128, 128], BF, tag=f"v{j}", name=f"v{j}")
                    for j in range(NKT)]
            xT_sb = [persist.tile([128, S], BF, tag=f"xT{h}", name=f"xT{h}")
                     for h in range(G)]
            kT = qk_sb[G]

            # rows: m 0..3 -> q head m (RoPE), 4 -> k (RoPE), 5 -> v (plain)
            def row_bias(m, ps, cs):
                """PSUM->SBUF copy with bias; returns rope tmp or None."""
                if m == ROWS - 1:
                    nc.scalar.activation(vT_sb[:, cs], ps, IDF,
                                         bias=bias_sb[:, m:m + 1])
                    return None
                tmp = work.tile([128, SQ], BF, tag="tmp", name="tmp")
                nc.scalar.activation(tmp, ps, IDF, bias=bias_sb[:, m:m + 1])
                return tmp

            def row_rope(m, tmp, cs):
                rp = psAV.tile([128, SQ], F32, tag="av", name="rp")
                nc.tensor.matmul(rp, rt_sb, tmp, start=True, stop=True)
                rot = work.tile([128, SQ], BF, tag="rot", name="rot")
                nc.vector.tensor_mul(rot, rp, sin_sb[:, cs])
                tcos = work.tile([128, SQ], BF, tag="tcos", name="tcos")
                nc.vector.tensor_mul(tcos, tmp, cos_sb[:, cs])
                nc.vector.tensor_add(qk_sb[min(m, G)][:, cs], rot, tcos)

            def oproj_chunk(c):
                for t in range(4 * c, 4 * c + 4):
                    ts_ = slice(t * 128, (t + 1) * 128)
                    osb = work2.tile([128, H], BF, tag="osb", name="osb")
                    for npair in range(2):
                        op = psS.tile([128, 2, SQ], F32, tag="sc", name="op")
                        for g in range(G):
                            for n2 in range(2):
                                n = 2 * npair + n2
                                nc.tensor.matmul(
                                    op[:, n2, :], xT_sb[g][:, ts_],
                                    wo_sb[g][:, n * SQ:(n + 1) * SQ],
                                    start=(g == 0), stop=(g == G - 1),
                                )
                        dst = osb[:, npair * 2 * SQ:(npair + 1) * 2 * SQ]
                        if npair == 0:
                            nc.scalar.copy(dst, op)
                        else:
                            nc.vector.tensor_copy(dst, op)
                    nc.sync.dma_start(out=out[ts_, :], in_=osb)

            def attn_chunk(c):
                cs = slice(c * SQ, (c + 1) * SQ)
                njt = 4 * c + 4
                for hp in range(2):
                    h0, h1 = 2 * hp, 2 * hp + 1
                    if hp == 0:
                        av0 = psAV.tile([128, SQ], F32, tag="av", name="av0")
                        av1 = psAV.tile([128, SQ], F32, tag="av", name="av1")
                    else:
                        av0 = psQ.tile([128, SQ], F32, tag="qkv", name="av0b")
                        av1 = psQ.tile([128, SQ], F32, tag="qkv", name="av1b")
                    dacc = work2.tile([128, 2, SQ], BF, tag="dacc", name="dacc")
                    pend = None  # (j, ex, off) awaiting its attn@V matmuls
                    for j in range(njt):
                        i = j - 4 * c
                        off = 128 * i if i > 0 else 0
                        w = SQ - off
                        sc = psS.tile([128, 2, SQ], F32, tag="sc", name="sc")
                        for hs, h in ((0, h0), (1, h1)):
                            nc.tensor.matmul(
                                sc[:, hs, off:],
                                kT[:, j * 128:(j + 1) * 128],
                                qk_sb[h][:, c * SQ + off:(c + 1) * SQ],
                                start=True, stop=True,
                            )
                        if pend is not None:
                            pj, pex, poff = pend
                            nc.tensor.matmul(av0[:, poff:], v_sb[pj],
                                             pex[:, 0, poff:],
                                             start=(pj == 0), stop=False)
                            nc.tensor.matmul(av1[:, poff:], v_sb[pj],
                                             pex[:, 1, poff:],
                                             start=(pj == 0), stop=False)
                        ex = work.tile([128, 2, SQ], BF, tag="ex", name="ex")
                        nc.scalar.activation(ex[:, :, off:], sc[:, :, off:],
                                             EXPF, scale=SCALE)
                        if i >= 0:
                            nc.vector.tensor_mul(ex[:, :, off:off + 128],
                                                 ex[:, :, off:off + 128],
                                                 mask_sb)
                        if j == 0:
                            nc.vector.tensor_copy(dacc, ex)
                        else:
                            eng = nc.vector if (j % 2 == 0) else nc.gpsimd
                            eng.tensor_add(dacc[:, :, off:], dacc[:, :, off:],
                                           ex[:, :, off:])
                        pend = (j, ex, off)
                    pj, pex, poff = pend
                    nc.tensor.matmul(av0[:, poff:], v_sb[pj], pex[:, 0, poff:],
                                     start=(pj == 0), stop=True)
                    nc.tensor.matmul(av1[:, poff:], v_sb[pj], pex[:, 1, poff:],
                                     start=(pj == 0), stop=True)
                    for hs, av in ((0, av0), (1, av1)):
                        h = 2 * hp + hs
                        dn = psS.tile([1, SQ], F32, tag="sc", name="dn")
                        nc.tensor.matmul(dn, ones_f, dacc[:, hs, :],
                                         start=True, stop=True)
                        rd = work2.tile([1, SQ], F32, tag="rd", name="rd")
                        nc.vector.reciprocal_approx_fast(rd, dn)
                        rdb = work2.tile([128, SQ], F32, tag="rdb", name="rdb")
                        nc.gpsimd.partition_broadcast(rdb, rd)
                        nc.vector.tensor_mul(xT_sb[h][:, cs], av, rdb)

            ROW_ORDER = (G, ROWS - 1, 0, 1, 2, 3)  # k, v, q0..q3
            for c in range(NQC):
                cs = slice(c * SQ, (c + 1) * SQ)
                # prefetch next chunk's hidden tiles
                if c + 1 < NQC:
                    for kt in range(NHT):
                        ht = hbuf.tile([128, SQ], BF, tag=f"h{kt}",
                                       name=f"h{c + 1}_{kt}")
                        nc.sync.dma_start(
                            out=ht,
                            in_=hT[kt * 128:(kt + 1) * 128,
                                   (c + 1) * SQ:(c + 2) * SQ])
                        h_tiles[c + 1][kt] = ht
                # ---- QKV projection + RoPE ----
                if c == 0:
                    # k-tile-outer so compute starts as DMA streams in;
                    # 6 concurrent accumulators spread over all three pools
                    pools = {0: psQ, 1: psQ, 2: psS, 3: psS, 4: psAV, 5: psAV}
                    tags = {0: "qkv", 1: "qkv", 2: "sc", 3: "sc",
                            4: "av", 5: "av"}
                    accs = {m: pools[m].tile([128, SQ], F32, tag=tags[m],
                                             name=f"acc{m}")
                            for m in range(ROWS)}
                    for kt in range(NHT):
                        for m in range(ROWS):
                            nc.tensor.matmul(
                                accs[m],
                                wq_sb[kt][:, m * 128:(m + 1) * 128],
                                h_tiles[0][kt],
                                start=(kt == 0), stop=(kt == NHT - 1),
                            )
                    tmps = {m: row_bias(m, accs[m], cs) for m in ROW_ORDER}
                    for m in ROW_ORDER:
                        if tmps[m] is not None:
                            row_rope(m, tmps[m], cs)
                else:
                    prev = None  # stagger rope behind next row's matmuls
                    for m in ROW_ORDER:
                        ps = psQ.tile([128, SQ], F32, tag="qkv", name="mm")
                        for kt in range(NHT):
                            nc.tensor.matmul(
                                ps,
                                wq_sb[kt][:, m * 128:(m + 1) * 128],
                                h_tiles[c][kt],
                                start=(kt == 0), stop=(kt == NHT - 1),
                            )
                        if prev is not None:
                            row_rope(prev[0], prev[1], cs)
                            prev = None
                        tmp = row_bias(m, ps, cs)
                        if tmp is not None:
                            prev = (m, tmp)
                    if prev is not None:
                        row_rope(prev[0], prev[1], cs)
                # ---- output projection of previous chunk ----
                if c > 0:
                    oproj_chunk(c - 1)
                # ---- transpose this chunk's v tiles ----
                for j in range(4 * c, 4 * c + 4):
                    tp = psS.tile([128, 128], BF, tag="sc", name="tp")
                    nc.tensor.transpose(tp, vT_sb[:, j * 128:(j + 1) * 128],
                                        id_sb)
                    nc.scalar.copy(v_sb[j], tp)
                # ---- attention ----
                attn_chunk(c)
            oproj_chunk(NQC - 1)
    nc.compile()
    return nc


def make_in_maps(hidden_states, cos, sin, Wq, bq, Wk, bk, Wv, bv, Wo, bo):
    """Host-side shard/pack. Returns list of 8 input dicts."""
    f32 = np.float32
    cosT = np.ascontiguousarray(np.asarray(cos).T).astype(BF16)
    sinT = np.ascontiguousarray(np.asarray(sin).T).astype(BF16)
    R = np.zeros((128, 128), f32)
    for d in range(64):
        R[d, d + 64] = -1.0
        R[d + 64, d] = 1.0
    rotT = np.ascontiguousarray(R.T).astype(BF16)
    # triangular mask for the diagonal 128-block, duplicated per head-pair
    p = np.arange(128)[:, None]
    q = np.arange(128)[None, :]
    tri = (q >= p).astype(BF16)
    masks2 = np.concatenate([tri, tri], axis=1)  # [128, 256]
    id128 = np.eye(128, dtype=BF16)

    in_maps = []
    for core in range(N_CORES):
        b, k = core // 4, core % 4
        hTc = np.ascontiguousarray(np.asarray(hidden_states[b]).T).astype(BF16)
        wq = Wq[512 * k:512 * (k + 1)]            # [512, H]
        wk = Wk[128 * k:128 * (k + 1)]            # [128, H]
        wv = Wv[128 * k:128 * (k + 1)]
        wqkvT = np.ascontiguousarray(
            np.concatenate([wq, wk, wv], axis=0).T
        ).astype(BF16)                             # [H, 768]
        bqkv = np.concatenate(
            [bq[512 * k:512 * (k + 1)], bk[128 * k:128 * (k + 1)],
             bv[128 * k:128 * (k + 1)]]
        ).astype(f32).reshape(ROWS, 128).T.copy()  # [128, ROWS]
        woT = np.ascontiguousarray(Wo[:, 512 * k:512 * (k + 1)].T).astype(BF16)
        in_maps.append({
            "hT": hTc, "wqkvT": wqkvT, "bqkv": bqkv,
            "cosT": cosT, "sinT": sinT, "masks2": masks2, "rotT": rotT,
            "woT": woT, "id128": id128,
        })
    return in_maps


_NC = None


def kernel(**inputs) -> np.ndarray:
    global _NC
    from concourse.bass_utils import run_bass_kernel_spmd

    if _NC is None:
        _NC = build_nc()
    in_maps = make_in_maps(**inputs)
    res = run_bass_kernel_spmd(_NC, in_maps, core_ids=list(range(N_CORES)))
    out = np.zeros((B, S, H), np.float32)
    for core in range(N_CORES):
        out[core // 4] += np.asarray(res.results[core]["out"], np.float32)
    out += np.asarray(inputs["bo"], np.float32)
    return out


# revision 3
# speedup vs baseline: 1.5274x; 1.4388x over previous
"""GQA kernel for Trainium2, sharded over 8 NeuronCores.

Problem: B=2, S=2048, H=2048, NH=16 q-heads, KVH=4 kv-heads, D=128.
Sharding: core c -> (batch b = c//4, kv-head k = c%4). Each core computes the
full attention for its 4 query heads + its kv head on its batch, plus the
row-parallel partial of the output projection. Host sums the 4 partials per
batch and adds the output bias.

v2 design (single fused pass per 512-token q-chunk):
  for c in 0..3:
    prefetch h(c+1); QKV projection + RoPE for chunk c (rows k,v,q0..q3);
    oproj for chunk c-1; transpose v tiles of chunk c;
    attention for chunk c in two head-pair sweeps:
      per kv tile j: 2 score matmuls (kT[j] stationary), ONE exp over the
      [128, 2, w] head-pair mega-tile (causally trimmed width w), triangular
      mask only on the 128-wide diagonal block, denominator accumulated in
      bf16 alternating DVE/GPSIMD, attn@V accumulated in PSUM;
      per head: ones-matmul partition-reduce -> reciprocal_approx_fast ->
      gpsimd partition_broadcast -> normalize into xT (bf16).
  oproj chunk 3.
Output partials written as bf16; host upcasts, sums, and adds bo.
"""

import numpy as np
import ml_dtypes

import concourse.bass as bass
import concourse.mybir as mybir
import concourse.tile as tile
from concourse import bacc

BF16 = ml_dtypes.bfloat16
F32 = mybir.dt.float32
BF = mybir.dt.bfloat16

B, S, H = 2, 2048, 2048
NH, KVH, D = 16, 4, 128
G = NH // KVH  # q heads per kv head / per core
N_CORES = 8
SCALE = 1.0 / float(np.sqrt(D))

SQ = 512              # q-chunk width
NQC = S // SQ         # 4 q chunks
NKT = S // 128        # 16 kv tiles / token tiles
NHT = H // 128        # 16 hidden k-tiles
ROWS = G + 2          # 6 projection row-blocks: 4 q heads, k, v
EXPF = mybir.ActivationFunctionType.Exp
IDF = mybir.ActivationFunctionType.Identity


def build_nc(num_devices: int = N_CORES) -> bass.Bass:
    nc = bacc.Bacc("TRN2", num_devices=num_devices)

    hT = nc.dram_tensor("hT", [H, S], BF, kind="ExternalInput").ap()
    wqkvT = nc.dram_tensor("wqkvT", [H, ROWS * 128], BF, kind="ExternalInput").ap()
    bqkv = nc.dram_tensor("bqkv", [128, ROWS], F32, kind="ExternalInput").ap()
    cosT = nc.dram_tensor("cosT", [128, S], BF, kind="ExternalInput").ap()
    sinT = nc.dram_tensor("sinT", [128, S], BF, kind="ExternalInput").ap()
    rotT = nc.dram_tensor("rotT", [128, 128], BF, kind="ExternalInput").ap()
    masks2 = nc.dram_tensor("masks2", [128, 256], BF, kind="ExternalInput").ap()
    woT = nc.dram_tensor("woT", [G * 128, H], BF, kind="ExternalInput").ap()
    id128 = nc.dram_tensor("id128", [128, 128], BF, kind="ExternalInput").ap()
    out = nc.dram_tensor("out", [S, H], BF, kind="ExternalOutput").ap()

    with tile.TileContext(nc) as tc:
        with (
            tc.tile_pool(name="consts", bufs=1) as consts,
            tc.tile_pool(name="persist", bufs=1) as persist,
            tc.tile_pool(name="hbuf", bufs=2) as hbuf,
            tc.tile_pool(name="work", bufs=3) as work,
            tc.tile_pool(name="work2", bufs=2) as work2,
            tc.tile_pool(name="psQ", bufs=2, space="PSUM") as psQ,
            tc.tile_pool(name="psS", bufs=2, space="PSUM") as psS,
            tc.tile_pool(name="psAV", bufs=2, space="PSUM") as psAV,
        ):
            # ---- small constants (first in DMA queue) ----
            bias_sb = consts.tile([128, ROWS], F32, tag="bias", name="bias")
            nc.sync.dma_start(out=bias_sb, in_=bqkv)
            rt_sb = consts.tile([128, 128], BF, tag="rt", name="rt")
            nc.sync.dma_start(out=rt_sb, in_=rotT)
            mask_sb = consts.tile([128, 2, 128], BF, tag="mask", name="mask")
            nc.sync.dma_start(out=mask_sb, in_=masks2)
            id_sb = consts.tile([128, 128], BF, tag="id", name="id")
            nc.sync.dma_start(out=id_sb, in_=id128)
            ones_f = consts.tile([128, 1], BF, tag="ones_f", name="ones_f")
            nc.vector.memset(ones_f, 1.0)
            # preload the exp activation table while DMA streams
            warm_in = consts.tile([128, 1], F32, tag="warm_in", name="warm_in")
            nc.vector.memset(warm_in, 0.0)
            warm_out = consts.tile([128, 1], BF, tag="warm_out", name="warm_out")
            nc.scalar.activation(warm_out, warm_in, EXPF)

            cos_sb = persist.tile([128, S], BF, tag="cos", name="cos")
            sin_sb = persist.tile([128, S], BF, tag="sin", name="sin")
            nc.sync.dma_start(out=cos_sb[:, 0:SQ], in_=cosT[:, 0:SQ])
            nc.sync.dma_start(out=sin_sb[:, 0:SQ], in_=sinT[:, 0:SQ])

            # ---- weights + chunk-0 hidden, interleaved by k-tile ----
            wq_sb = []
            h_tiles = [[None] * NHT for _ in range(NQC)]
            for kt in range(NHT):
                t = persist.tile([128, ROWS * 128], BF, tag=f"wq{kt}", name=f"wq{kt}")
                nc.sync.dma_start(out=t, in_=wqkvT[kt * 128:(kt + 1) * 128, :])
                wq_sb.append(t)
                ht = hbuf.tile([128, SQ], BF, tag=f"h{kt}", name=f"h0_{kt}")
                nc.sync.dma_start(out=ht, in_=hT[kt * 128:(kt + 1) * 128, 0:SQ])
                h_tiles[0][kt] = ht
            for c in range(1, NQC):
                cs = slice(c * SQ, (c + 1) * SQ)
                nc.sync.dma_start(out=cos_sb[:, cs], in_=cosT[:, cs])
                nc.sync.dma_start(out=sin_sb[:, cs], in_=sinT[:, cs])
            wo_sb = []
            for g in range(G):
                t = persist.tile([128, H], BF, tag=f"wo{g}", name=f"wo{g}")
                nc.sync.dma_start(out=t, in_=woT[g * 128:(g + 1) * 128, :])
                wo_sb.append(t)

            # persistent activations (bf16)
            qk_sb = [persist.tile([128, S], BF, tag=f"qk{m}", name=f"qk{m}")
                     for m in range(G + 1)]  # 0..3 q heads, 4 = k
            vT_sb = persist.tile([128, S], BF, tag="vT", name="vT")
            v_sb = [persist.tile([128, 128], BF, tag=f"v{j}", name=f"v{j}")
                    for j in range(NKT)]
            xT_sb = [persist.tile([128, S], BF, tag=f"xT{h}", name=f"xT{h}")
                     for h in range(G)]
            kT = qk_sb[G]

            # rows: m 0..3 -> q head m (RoPE), 4 -> k (RoPE), 5 -> v (plain)
            def row_bias(m, ps, cs):
                """PSUM->SBUF copy with bias; returns rope tmp or None."""
                if m == ROWS - 1:
                    nc.scalar.activation(vT_sb[:, cs], ps, IDF,
                                         bias=bias_sb[:, m:m + 1])
                    return None
                tmp = work.tile([128, SQ], BF, tag="tmp", name="tmp")
                nc.scalar.activation(tmp, ps, IDF, bias=bias_sb[:, m:m + 1])
                return tmp

            def row_rope(m, tmp, cs):
                rp = psAV.tile([128, SQ], F32, tag="av", name="rp")
                nc.tensor.matmul(rp, rt_sb, tmp, start=True, stop=True)
                rot = work.tile([128, SQ], BF, tag="rot", name="rot")
                nc.vector.tensor_mul(rot, rp, sin_sb[:, cs])
                tcos = work.tile([128, SQ], BF, tag="tcos", name="tcos")
                nc.vector.tensor_mul(tcos, tmp, cos_sb[:, cs])
                nc.vector.tensor_add(qk_sb[min(m, G)][:, cs], rot, tcos)

            def oproj_chunk(c):
                for t in range(4 * c, 4 * c + 4):
                    ts_ = slice(t * 128, (t + 1) * 128)
                    osb = work2.tile([128, H], BF, tag="osb", name="osb")
                    for npair in range(2):
                        op = psS.tile([128, 2, SQ], F32, tag="sc", name="op")
                        for g in range(G):
                            for n2 in range(2):
                                n = 2 * npair + n2
                                nc.tensor.matmul(
                                    op[:, n2, :], xT_sb[g][:, ts_],
                                    wo_sb[g][:, n * SQ:(n + 1) * SQ],
                                    start=(g == 0), stop=(g == G - 1),
                                )
                        dst = osb[:, npair * 2 * SQ:(npair + 1) * 2 * SQ]
                        if npair == 0:
                            nc.scalar.copy(dst, op)
                        else:
                            nc.vector.tensor_copy(dst, op)
                    nc.sync.dma_start(out=out[ts_, :], in_=osb)

            def attn_chunk(c):
                cs = slice(c * SQ, (c + 1) * SQ)
                njt = 4 * c + 4
                split = njt >= 8  # even/odd denominator chains (j=0,1 full)
                norms = []
                for hp in range(2):
                    h0, h1 = 2 * hp, 2 * hp + 1
                    if hp == 0:
                        av0 = psAV.tile([128, SQ], F32, tag="av", name="av0")
                        av1 = psAV.tile([128, SQ], F32, tag="av", name="av1")
                    else:
                        av0 = psQ.tile([128, SQ], F32, tag="qkv", name="av0b")
                        av1 = psQ.tile([128, SQ], F32, tag="qkv", name="av1b")
                    daccs = [work2.tile([128, 2, SQ], BF, tag=f"dacc{p}",
                                        name=f"dacc{p}")
                             for p in range(2 if split else 1)]
                    pend = None  # (j, ex, off) awaiting its attn@V matmuls
                    for j in range(njt):
                        i = j - 4 * c
                        off = 128 * i if i > 0 else 0
                        sc = psS.tile([128, 2, SQ], F32, tag="sc", name="sc")
                        for hs, h in ((0, h0), (1, h1)):
                            nc.tensor.matmul(
                                sc[:, hs, off:],
                                kT[:, j * 128:(j + 1) * 128],
                                qk_sb[h][:, c * SQ + off:(c + 1) * SQ],
                                start=True, stop=True,
                            )
                        if pend is not None:
                            pj, pex, poff = pend
                            nc.tensor.matmul(av0[:, poff:], v_sb[pj],
                                             pex[:, 0, poff:],
                                             start=(pj == 0), stop=False)
                            nc.tensor.matmul(av1[:, poff:], v_sb[pj],
                                             pex[:, 1, poff:],
                                             start=(pj == 0), stop=False)
                        ex = work.tile([128, 2, SQ], BF, tag="ex", name="ex")
                        nc.scalar.activation(ex[:, :, off:], sc[:, :, off:],
                                             EXPF, scale=SCALE)
                        if i >= 0:
                            nc.vector.tensor_mul(ex[:, :, off:off + 128],
                                                 ex[:, :, off:off + 128],
                                                 mask_sb)
                        dacc = daccs[j % 2] if split else daccs[0]
                        if j < (2 if split else 1):
                            nc.vector.tensor_copy(dacc, ex)
                        else:
                            nc.vector.tensor_add(dacc[:, :, off:],
                                                 dacc[:, :, off:],
                                                 ex[:, :, off:])
                        pend = (j, ex, off)
                    pj, pex, poff = pend
                    nc.tensor.matmul(av0[:, poff:], v_sb[pj], pex[:, 0, poff:],
                                     start=(pj == 0), stop=True)
                    nc.tensor.matmul(av1[:, poff:], v_sb[pj], pex[:, 1, poff:],
                                     start=(pj == 0), stop=True)

                    def norm(hp=hp, av0=av0, av1=av1, daccs=daccs):
                        for hs, av in ((0, av0), (1, av1)):
                            h = 2 * hp + hs
                            dn = psS.tile([1, SQ], F32, tag="sc", name="dn")
                            for p, dacc in enumerate(daccs):
                                nc.tensor.matmul(
                                    dn, ones_f, dacc[:, hs, :],
                                    start=(p == 0),
                                    stop=(p == len(daccs) - 1))
                            rd = work2.tile([1, SQ], F32, tag="rd", name="rd")
                            nc.vector.reciprocal_approx_fast(rd, dn)
                            rdb = work2.tile([128, SQ], F32, tag="rdb",
                                             name="rdb")
                            nc.gpsimd.partition_broadcast(rdb, rd)
                            nc.vector.tensor_mul(xT_sb[h][:, cs], av, rdb)
                    norms.append(norm)
                    if hp == 1:  # hp0's norm lands behind hp1's sweep
                        norms[0]()
                        norms[1]()

            ROW_ORDER = (G, ROWS - 1, 0, 1, 2, 3)  # k, v, q0..q3
            for c in range(NQC):
                cs = slice(c * SQ, (c + 1) * SQ)
                # prefetch next chunk's hidden tiles
                if c + 1 < NQC:
                    for kt in range(NHT):
                        ht = hbuf.tile([128, SQ], BF, tag=f"h{kt}",
                                       name=f"h{c + 1}_{kt}")
                        nc.sync.dma_start(
                            out=ht,
                            in_=hT[kt * 128:(kt + 1) * 128,
                                   (c + 1) * SQ:(c + 2) * SQ])
                        h_tiles[c + 1][kt] = ht
                # ---- QKV projection + RoPE ----
                if c == 0:
                    # k-tile-outer so compute starts as DMA streams in;
                    # 6 concurrent accumulators spread over all three pools
                    pools = {0: psQ, 1: psQ, 2: psS, 3: psS, 4: psAV, 5: psAV}
                    tags = {0: "qkv", 1: "qkv", 2: "sc", 3: "sc",
                            4: "av", 5: "av"}
                    accs = {m: pools[m].tile([128, SQ], F32, tag=tags[m],
                                             name=f"acc{m}")
                            for m in range(ROWS)}
                    for kt in range(NHT):
                        for m in range(ROWS):
                            nc.tensor.matmul(
                                accs[m],
                                wq_sb[kt][:, m * 128:(m + 1) * 128],
                                h_tiles[0][kt],
                                start=(kt == 0), stop=(kt == NHT - 1),
                            )
                    tmps = {m: row_bias(m, accs[m], cs) for m in ROW_ORDER}
                    for m in ROW_ORDER:
                        if tmps[m] is not None:
                            row_rope(m, tmps[m], cs)
                else:
                    prev = None  # stagger rope behind next row's matmuls
                    for m in ROW_ORDER:
                        ps = psQ.tile([128, SQ], F32, tag="qkv", name="mm")
                        for kt in range(NHT):
                            nc.tensor.matmul(
                                ps,
                                wq_sb[kt][:, m * 128:(m + 1) * 128],
                                h_tiles[c][kt],
                                start=(kt == 0), stop=(kt == NHT - 1),
                            )
                        if prev is not None:
                            row_rope(prev[0], prev[1], cs)
                            prev = None
                        tmp = row_bias(m, ps, cs)
                        if tmp is not None:
                            prev = (m, tmp)
                    if prev is not None:
                        row_rope(prev[0], prev[1], cs)
                # ---- output projection of previous chunk ----
                if c > 0:
                    oproj_chunk(c - 1)
                # ---- transpose this chunk's v tiles ----
                for j in range(4 * c, 4 * c + 4):
                    tp = psS.tile([128, 128], BF, tag="sc", name="tp")
                    nc.tensor.transpose(tp, vT_sb[:, j * 128:(j + 1) * 128],
                                        id_sb)
                    nc.scalar.copy(v_sb[j], tp)
                # ---- attention ----
                attn_chunk(c)
            oproj_chunk(NQC - 1)
    nc.compile()
    return nc


def make_in_maps(hidden_states, cos, sin, Wq, bq, Wk, bk, Wv, bv, Wo, bo):
    """Host-side shard/pack. Returns list of 8 input dicts."""
    f32 = np.float32
    cosT = np.ascontiguousarray(np.asarray(cos).T).astype(BF16)
    sinT = np.ascontiguousarray(np.asarray(sin).T).astype(BF16)
    R = np.zeros((128, 128), f32)
    for d in range(64):
        R[d, d + 64] = -1.0
        R[d + 64, d] = 1.0
    rotT = np.ascontiguousarray(R.T).astype(BF16)
    # triangular mask for the diagonal 128-block, duplicated per head-pair
    p = np.arange(128)[:, None]
    q = np.arange(128)[None, :]
    tri = (q >= p).astype(BF16)
    masks2 = np.concatenate([tri, tri], axis=1)  # [128, 256]
    id128 = np.eye(128, dtype=BF16)

    in_maps = []
    for core in range(N_CORES):
        b, k = core // 4, core % 4
        hTc = np.ascontiguousarray(np.asarray(hidden_states[b]).T).astype(BF16)
        wq = Wq[512 * k:512 * (k + 1)]            # [512, H]
        wk = Wk[128 * k:128 * (k + 1)]            # [128, H]
        wv = Wv[128 * k:128 * (k + 1)]
        wqkvT = np.ascontiguousarray(
            np.concatenate([wq, wk, wv], axis=0).T
        ).astype(BF16)                             # [H, 768]
        bqkv = np.concatenate(
            [bq[512 * k:512 * (k + 1)], bk[128 * k:128 * (k + 1)],
             bv[128 * k:128 * (k + 1)]]
        ).astype(f32).reshape(ROWS, 128).T.copy()  # [128, ROWS]
        woT = np.ascontiguousarray(Wo[:, 512 * k:512 * (k + 1)].T).astype(BF16)
        in_maps.append({
            "hT": hTc, "wqkvT": wqkvT, "bqkv": bqkv,
            "cosT": cosT, "sinT": sinT, "masks2": masks2, "rotT": rotT,
            "woT": woT, "id128": id128,
        })
    return in_maps


_NC = None


def kernel(**inputs) -> np.ndarray:
    global _NC
    from concourse.bass_utils import run_bass_kernel_spmd

    if _NC is None:
        _NC = build_nc()
    in_maps = make_in_maps(**inputs)
    res = run_bass_kernel_spmd(_NC, in_maps, core_ids=list(range(N_CORES)))
    out = np.zeros((B, S, H), np.float32)
    for core in range(N_CORES):
        out[core // 4] += np.asarray(res.results[core]["out"], np.float32)
    out += np.asarray(inputs["bo"], np.float32)
    return out


# revision 7
# speedup vs baseline: 1.6709x; 1.0939x over previous
"""GQA kernel for Trainium2, sharded over 8 NeuronCores.

Problem: B=2, S=2048, H=2048, NH=16 q-heads, KVH=4 kv-heads, D=128.
Sharding: core c -> (batch b = c//4, kv-head k = c%4). Each core computes the
full attention for its 4 query heads + its kv head on its batch, plus the
row-parallel partial of the output projection. Host sums the 4 partials per
batch and adds the output bias.

v4 design (single fused pass per 512-token q-chunk):
  for c in 0..3:
    prefetch h(c+1); QKV projection + RoPE for chunk c (rows k,v,q0..q3,
    k/v accumulated in the scores pool to decouple from attention slots);
    transpose v tiles of chunk c;
    attention for chunk c in two head-pair sweeps with oproj(c-1) micro-ops
    (one [128,512] output-column block: 4 accumulated matmuls + copy)
    interleaved between kv-tile iterations to fill the exp-wait bubbles:
      per kv tile j: 2 score matmuls (kT[j] stationary), ONE exp over the
      [128, 2, w] head-pair mega-tile (causally trimmed width w), triangular
      mask on the 128-wide diagonal block only, denominator accumulated in
      bf16 on DVE as independent even/odd chains, attn@V accumulated in PSUM
      (lagging one j so the matmul never waits on exp);
      per head: two accumulated ones-matmuls merge the chains ->
      reciprocal_approx_fast -> gpsimd partition_broadcast -> normalize into
      xT (bf16).
  trailing oproj(3).
All DMA is batched into a handful of large transfers (the chunk-0 weight and
hidden loads are quartered so compute starts while DMA streams). Output
partials are written as bf16; host upcasts, sums, and adds bo.
"""

import numpy as np
import ml_dtypes

import concourse.bass as bass
import concourse.mybir as mybir
import concourse.tile as tile
from concourse import bacc

BF16 = ml_dtypes.bfloat16
F32 = mybir.dt.float32
BF = mybir.dt.bfloat16

B, S, H = 2, 2048, 2048
NH, KVH, D = 16, 4, 128
G = NH // KVH  # q heads per kv head / per core
N_CORES = 8
SCALE = 1.0 / float(np.sqrt(D))

SQ = 512              # q-chunk width
NQC = S // SQ         # 4 q chunks
NKT = S // 128        # 16 kv tiles / token tiles
NHT = H // 128        # 16 hidden k-tiles
ROWS = G + 2          # 6 projection row-blocks: 4 q heads, k, v
EXPF = mybir.ActivationFunctionType.Exp
IDF = mybir.ActivationFunctionType.Identity


def build_nc(num_devices: int = N_CORES) -> bass.Bass:
    nc = bacc.Bacc("TRN2", num_devices=num_devices)

    # packed layouts: [partition, tile, cols] so one DMA moves many tiles
    hTd = nc.dram_tensor("hTd", [128, NHT, S], BF, kind="ExternalInput").ap()
    wqd = nc.dram_tensor("wqd", [128, NHT, ROWS * 128], BF,
                         kind="ExternalInput").ap()
    bqkv = nc.dram_tensor("bqkv", [128, ROWS], F32, kind="ExternalInput").ap()
    cosT = nc.dram_tensor("cosT", [128, S], BF, kind="ExternalInput").ap()
    sinT = nc.dram_tensor("sinT", [128, S], BF, kind="ExternalInput").ap()
    rotT = nc.dram_tensor("rotT", [128, 128], BF, kind="ExternalInput").ap()
    masks2 = nc.dram_tensor("masks2", [128, 256], BF, kind="ExternalInput").ap()
    wod = nc.dram_tensor("wod", [128, G, H], BF, kind="ExternalInput").ap()
    id128 = nc.dram_tensor("id128", [128, 128], BF, kind="ExternalInput").ap()
    out = nc.dram_tensor("out", [S, H], BF, kind="ExternalOutput").ap()

    with tile.TileContext(nc) as tc:
        with (
            tc.tile_pool(name="consts", bufs=1) as consts,
            tc.tile_pool(name="persist", bufs=1) as persist,
            tc.tile_pool(name="hbuf", bufs=2) as hbuf,
            tc.tile_pool(name="work", bufs=3) as work,
            tc.tile_pool(name="work2", bufs=2) as work2,
            tc.tile_pool(name="obuf", bufs=3) as obuf,
            tc.tile_pool(name="psQ", bufs=2, space="PSUM") as psQ,
            tc.tile_pool(name="psS", bufs=2, space="PSUM") as psS,
            tc.tile_pool(name="psAV", bufs=2, space="PSUM") as psAV,
        ):
            # ---- small constants (first in DMA queue) ----
            bias_sb = consts.tile([128, ROWS], F32, tag="bias", name="bias")
            nc.sync.dma_start(out=bias_sb, in_=bqkv)
            rt_sb = consts.tile([128, 128], BF, tag="rt", name="rt")
            nc.sync.dma_start(out=rt_sb, in_=rotT)
            mask_sb = consts.tile([128, 2, 128], BF, tag="mask", name="mask")
            nc.sync.dma_start(out=mask_sb, in_=masks2)
            id_sb = consts.tile([128, 128], BF, tag="id", name="id")
            nc.sync.dma_start(out=id_sb, in_=id128)
            ones_f = consts.tile([128, 1], BF, tag="ones_f", name="ones_f")
            nc.vector.memset(ones_f, 1.0)
            # preload the exp activation table while DMA streams
            warm_in = consts.tile([128, 1], F32, tag="warm_in", name="warm_in")
            nc.vector.memset(warm_in, 0.0)
            warm_out = consts.tile([128, 1], BF, tag="warm_out", name="warm_out")
            nc.scalar.activation(warm_out, warm_in, EXPF)

            # weights + chunk-0 hidden: quartered along k-tiles so the first
            # matmuls start after ~1.3MB instead of 5.2MB
            wq_q, h_q = [], [[None] * 4 for _ in range(NQC)]
            for q in range(4):
                wt = persist.tile([128, 4, ROWS * 128], BF, tag=f"wq{q}",
                                  name=f"wq{q}")
                nc.sync.dma_start(out=wt, in_=wqd[:, 4 * q:4 * q + 4, :])
                wq_q.append(wt)
                ht = hbuf.tile([128, 4, SQ], BF, tag=f"h{q}", name=f"h0_{q}")
                nc.sync.dma_start(out=ht, in_=hTd[:, 4 * q:4 * q + 4, 0:SQ])
                h_q[0][q] = ht
            cos_sb = persist.tile([128, S], BF, tag="cos", name="cos")
            nc.sync.dma_start(out=cos_sb, in_=cosT)
            sin_sb = persist.tile([128, S], BF, tag="sin", name="sin")
            nc.sync.dma_start(out=sin_sb, in_=sinT)
            wo_sb = persist.tile([128, G, H], BF, tag="wo", name="wo")
            nc.sync.dma_start(out=wo_sb, in_=wod)

            def wq_ap(kt, m):
                return wq_q[kt // 4][:, kt % 4, m * 128:(m + 1) * 128]

            def h_ap(c, kt):
                return h_q[c][kt // 4][:, kt % 4, :]

            # persistent activations (bf16)
            qk_sb = [persist.tile([128, S], BF, tag=f"qk{m}", name=f"qk{m}")
                     for m in range(G + 1)]  # 0..3 q heads, 4 = k
            vT_sb = persist.tile([128, S], BF, tag="vT", name="vT")
            v_sb = [persist.tile([128, 128], BF, tag=f"v{j}", name=f"v{j}")
                    for j in range(NKT)]
            xT_sb = [persist.tile([128, S], BF, tag=f"xT{h}", name=f"xT{h}")
                     for h in range(G)]
            kT = qk_sb[G]

            # rows: m 0..3 -> q head m (RoPE), 4 -> k (RoPE), 5 -> v (plain)
            def row_bias(m, ps, cs):
                """PSUM->SBUF copy with bias; returns rope tmp or None."""
                if m == ROWS - 1:
                    nc.scalar.activation(vT_sb[:, cs], ps, IDF,
                                         bias=bias_sb[:, m:m + 1])
                    return None
                tmp = work.tile([128, SQ], BF, tag="tmp", name="tmp")
                nc.scalar.activation(tmp, ps, IDF, bias=bias_sb[:, m:m + 1])
                return tmp

            def row_rope(m, tmp, cs):
                rp = psAV.tile([128, SQ], F32, tag="av", name="rp")
                nc.tensor.matmul(rp, rt_sb, tmp, start=True, stop=True)
                rot = work.tile([128, SQ], BF, tag="rot", name="rot")
                nc.vector.tensor_mul(rot, rp, sin_sb[:, cs])
                tcos = work.tile([128, SQ], BF, tag="tcos", name="tcos")
                nc.vector.tensor_mul(tcos, tmp, cos_sb[:, cs])
                nc.vector.tensor_add(qk_sb[min(m, G)][:, cs], rot, tcos)

            def oproj_fill_ops(c):
                """One micro-op per (token tile, output column block): alloc a
                PSUM tile, 4 accumulated matmuls over heads, copy into the
                per-tile output staging buffer, DMA the row block when done.
                Each closure takes the PSUM pool to use (the one that is free
                during the sweep it is interleaved into)."""
                osbs = {}
                ops = []
                for ti, t in enumerate(range(4 * c, 4 * c + 4)):
                    for n in range(G):
                        def op_fn(pool, t=t, n=n):
                            if t not in osbs:
                                osbs[t] = obuf.tile([128, H], BF, tag="osb",
                                                    name="osb")
                            osb = osbs[t]
                            op = pool.tile([128, SQ], F32,
                                           tag="qkv" if pool is psQ else "av",
                                           name="op")
                            ts_ = slice(t * 128, (t + 1) * 128)
                            for g in range(G):
                                nc.tensor.matmul(
                                    op, xT_sb[g][:, ts_],
                                    wo_sb[:, g, n * SQ:(n + 1) * SQ],
                                    start=(g == 0), stop=(g == G - 1))
                            dst = osb[:, n * SQ:(n + 1) * SQ]
                            if n % 2 == 0:
                                nc.scalar.copy(dst, op)
                            else:
                                nc.vector.tensor_copy(dst, op)
                            if n == G - 1:
                                nc.sync.dma_start(
                                    out=out[t * 128:(t + 1) * 128, :], in_=osb)
                        ops.append(op_fn)
                return ops

            def attn_chunk(c, fill):
                cs = slice(c * SQ, (c + 1) * SQ)
                njt = 4 * c + 4
                split = njt >= 8  # even/odd denominator chains (j=0,1 full)
                for hp in range(2):
                    h0, h1 = 2 * hp, 2 * hp + 1
                    if hp == 0:
                        av0 = psAV.tile([128, SQ], F32, tag="av", name="av0")
                        av1 = psAV.tile([128, SQ], F32, tag="av", name="av1")
                    else:
                        av0 = psQ.tile([128, SQ], F32, tag="qkv", name="av0b")
                        av1 = psQ.tile([128, SQ], F32, tag="qkv", name="av1b")
                    daccs = [work2.tile([128, 2, SQ], BF, tag=f"dacc{p}",
                                        name=f"dacc{p}")
                             for p in range(2 if split else 1)]
                    pend = None  # (j, ex, off) awaiting its attn@V matmuls
                    for j in range(njt):
                        i = j - 4 * c
                        off = 128 * i if i > 0 else 0
                        sc = psS.tile([128, 2, SQ], F32, tag="sc", name="sc")
                        for hs, h in ((0, h0), (1, h1)):
                            nc.tensor.matmul(
                                sc[:, hs, off:],
                                kT[:, j * 128:(j + 1) * 128],
                                qk_sb[h][:, c * SQ + off:(c + 1) * SQ],
                                start=True, stop=True,
                            )
                        if pend is not None:
                            pj, pex, poff = pend
                            nc.tensor.matmul(av0[:, poff:], v_sb[pj],
                                             pex[:, 0, poff:],
                                             start=(pj == 0), stop=False)
                            nc.tensor.matmul(av1[:, poff:], v_sb[pj],
                                             pex[:, 1, poff:],
                                             start=(pj == 0), stop=False)
                        if j >= 1 and fill:
                            # psQ is free during hp0 (held by hp1's attn@V
                            # accumulators later); psAV frees once hp0's
                            # normalization has read av0/av1
                            fill.pop(0)(psQ if hp == 0 else psAV)
                        ex = work.tile([128, 2, SQ], BF, tag="ex", name="ex")
                        nc.scalar.activation(ex[:, :, off:], sc[:, :, off:],
                                             EXPF, scale=SCALE)
                        if i >= 0:
                            nc.vector.tensor_mul(ex[:, :, off:off + 128],
                                                 ex[:, :, off:off + 128],
                                                 mask_sb)
                        dacc = daccs[j % 2] if split else daccs[0]
                        if j < (2 if split else 1):
                            nc.vector.tensor_copy(dacc, ex)
                        else:
                            nc.vector.tensor_add(dacc[:, :, off:],
                                                 dacc[:, :, off:],
                                                 ex[:, :, off:])
                        pend = (j, ex, off)
                    pj, pex, poff = pend
                    nc.tensor.matmul(av0[:, poff:], v_sb[pj], pex[:, 0, poff:],
                                     start=(pj == 0), stop=True)
                    nc.tensor.matmul(av1[:, poff:], v_sb[pj], pex[:, 1, poff:],
                                     start=(pj == 0), stop=True)
                    # normalize both heads of the pair
                    for hs, av in ((0, av0), (1, av1)):
                        h = 2 * hp + hs
                        dn = psS.tile([1, SQ], F32, tag="sc", name="dn")
                        for p, dacc in enumerate(daccs):
                            nc.tensor.matmul(dn, ones_f, dacc[:, hs, :],
                                             start=(p == 0),
                                             stop=(p == len(daccs) - 1))
                        rd = work2.tile([1, SQ], F32, tag="rd", name="rd")
                        nc.vector.reciprocal_approx_fast(rd, dn)
                        rdb = work2.tile([128, SQ], F32, tag="rdb", name="rdb")
                        nc.gpsimd.partition_broadcast(rdb, rd)
                        nc.vector.tensor_mul(xT_sb[h][:, cs], av, rdb)

            ROW_ORDER = (G, ROWS - 1, 0, 1, 2, 3)  # k, v, q0..q3
            for c in range(NQC):
                cs = slice(c * SQ, (c + 1) * SQ)
                # prefetch next chunk's hidden tiles
                if c + 1 < NQC:
                    for q in range(4):
                        ht = hbuf.tile([128, 4, SQ], BF, tag=f"h{q}",
                                       name=f"h{c + 1}_{q}")
                        nc.sync.dma_start(
                            out=ht,
                            in_=hTd[:, 4 * q:4 * q + 4,
                                    (c + 1) * SQ:(c + 2) * SQ])
                        h_q[c + 1][q] = ht
                # ---- QKV projection + RoPE ----
                if c == 0:
                    # k-tile-outer so compute starts as DMA streams in;
                    # 6 concurrent accumulators spread over all three pools
                    pools = {0: psQ, 1: psQ, 2: psS, 3: psS, 4: psAV, 5: psAV}
                    tags = {0: "qkv", 1: "qkv", 2: "sc", 3: "sc",
                            4: "av", 5: "av"}
                    accs = {m: pools[m].tile([128, SQ], F32, tag=tags[m],
                                             name=f"acc{m}")
                            for m in range(ROWS)}
                    for kt in range(NHT):
                        for m in range(ROWS):
                            nc.tensor.matmul(
                                accs[m], wq_ap(kt, m), h_ap(0, kt),
                                start=(kt == 0), stop=(kt == NHT - 1),
                            )
                    tmps = {m: row_bias(m, accs[m], cs) for m in ROW_ORDER}
                    for m in ROW_ORDER:
                        if tmps[m] is not None:
                            row_rope(m, tmps[m], cs)
                else:
                    prev = None  # stagger rope behind next row's matmuls
                    for m in ROW_ORDER:
                        # k and v rows accumulate in the scores pool so they
                        # don't wait on the previous chunk's hp1 attention
                        pool, tag = (psS, "sc") if m >= G else (psQ, "qkv")
                        ps = pool.tile([128, SQ], F32, tag=tag, name="mm")
                        for kt in range(NHT):
                            nc.tensor.matmul(
                                ps, wq_ap(kt, m), h_ap(c, kt),
                                start=(kt == 0), stop=(kt == NHT - 1),
                            )
                        if prev is not None:
                            row_rope(prev[0], prev[1], cs)
                            prev = None
                        tmp = row_bias(m, ps, cs)
                        if tmp is not None:
                            prev = (m, tmp)
                    if prev is not None:
                        row_rope(prev[0], prev[1], cs)
                # ---- transpose this chunk's v tiles ----
                for j in range(4 * c, 4 * c + 4):
                    tp = psS.tile([128, 128], BF, tag="sc", name="tp")
                    nc.tensor.transpose(tp, vT_sb[:, j * 128:(j + 1) * 128],
                                        id_sb)
                    nc.scalar.copy(v_sb[j], tp)
                # ---- attention with oproj(c-1) interleaved ----
                fill = oproj_fill_ops(c - 1) if c > 0 else []
                attn_chunk(c, fill)
                for fi, fn in enumerate(fill):  # leftovers (c=1 only)
                    fn(psQ if fi % 2 else psAV)
            for fi, fn in enumerate(oproj_fill_ops(NQC - 1)):
                fn(psQ if fi % 2 else psAV)
    nc.compile()
    return nc


def make_in_maps(hidden_states, cos, sin, Wq, bq, Wk, bk, Wv, bv, Wo, bo):
    """Host-side shard/pack. Returns list of 8 input dicts."""
    f32 = np.float32
    cosT = np.ascontiguousarray(np.asarray(cos).T).astype(BF16)
    sinT = np.ascontiguousarray(np.asarray(sin).T).astype(BF16)
    R = np.zeros((128, 128), f32)
    for d in range(64):
        R[d, d + 64] = -1.0
        R[d + 64, d] = 1.0
    rotT = np.ascontiguousarray(R.T).astype(BF16)
    # triangular mask for the diagonal 128-block, duplicated per head-pair
    p = np.arange(128)[:, None]
    q = np.arange(128)[None, :]
    tri = (q >= p).astype(BF16)
    masks2 = np.concatenate([tri, tri], axis=1)  # [128, 256]
    id128 = np.eye(128, dtype=BF16)

    in_maps = []
    for core in range(N_CORES):
        b, k = core // 4, core % 4
        hT = np.ascontiguousarray(np.asarray(hidden_states[b]).T)  # [H, S]
        hTd = np.ascontiguousarray(
            hT.reshape(NHT, 128, S).transpose(1, 0, 2)).astype(BF16)
        wq = Wq[512 * k:512 * (k + 1)]            # [512, H]
        wk = Wk[128 * k:128 * (k + 1)]            # [128, H]
        wv = Wv[128 * k:128 * (k + 1)]
        wqkvT = np.ascontiguousarray(
            np.concatenate([wq, wk, wv], axis=0).T)  # [H, 768]
        wqd = np.ascontiguousarray(
            wqkvT.reshape(NHT, 128, ROWS * 128).transpose(1, 0, 2)
        ).astype(BF16)                             # [128, 16, 768]
        bqkv = np.concatenate(
            [bq[512 * k:512 * (k + 1)], bk[128 * k:128 * (k + 1)],
             bv[128 * k:128 * (k + 1)]]
        ).astype(f32).reshape(ROWS, 128).T.copy()  # [128, ROWS]
        woT = np.ascontiguousarray(Wo[:, 512 * k:512 * (k + 1)].T)  # [512, H]
        wod = np.ascontiguousarray(
            woT.reshape(G, 128, H).transpose(1, 0, 2)).astype(BF16)
        in_maps.append({
            "hTd": hTd, "wqd": wqd, "bqkv": bqkv,
            "cosT": cosT, "sinT": sinT, "masks2": masks2, "rotT": rotT,
            "wod": wod, "id128": id128,
        })
    return in_maps


_NC = None


def kernel(**inputs) -> np.ndarray:
    global _NC
    from concourse.bass_utils import run_bass_kernel_spmd

    if _NC is None:
        _NC = build_nc()
    in_maps = make_in_maps(**inputs)
    res = run_bass_kernel_spmd(_NC, in_maps, core_ids=list(range(N_CORES)))
    out = np.zeros((B, S, H), np.float32)
    for core in range(N_CORES):
        out[core // 4] += np.asarray(res.results[core]["out"], np.float32)
    out += np.asarray(inputs["bo"], np.float32)
    return out


# revision 13
# speedup vs baseline: 1.6773x; 1.0038x over previous
"""GQA kernel for Trainium2, sharded over 8 NeuronCores.

Problem: B=2, S=2048, H=2048, NH=16 q-heads, KVH=4 kv-heads, D=128.
Sharding: core c -> (batch b = c//4, kv-head k = c%4). Each core computes the
full attention for its 4 query heads + its kv head on its batch, plus the
row-parallel partial of the output projection. Host sums the 4 partials per
batch and adds the output bias.

v4 design (single fused pass per 512-token q-chunk):
  for c in 0..3:
    prefetch h(c+1); QKV projection + RoPE for chunk c (rows k,v,q0..q3,
    k/v accumulated in the scores pool to decouple from attention slots);
    transpose v tiles of chunk c;
    attention for chunk c in two head-pair sweeps with oproj(c-1) micro-ops
    (one [128,512] output-column block: 4 accumulated matmuls + copy)
    interleaved between kv-tile iterations to fill the exp-wait bubbles:
      per kv tile j: 2 score matmuls (kT[j] stationary), ONE exp over the
      [128, 2, w] head-pair mega-tile (causally trimmed width w), triangular
      mask on the 128-wide diagonal block only, denominator accumulated in
      bf16 on DVE as independent even/odd chains, attn@V accumulated in PSUM
      (lagging one j so the matmul never waits on exp);
      per head: two accumulated ones-matmuls merge the chains ->
      reciprocal_approx_fast -> gpsimd partition_broadcast -> normalize into
      xT (bf16).
  trailing oproj(3).
All DMA is batched into a handful of large transfers (the chunk-0 weight and
hidden loads are quartered so compute starts while DMA streams). Output
partials are written as bf16; host upcasts, sums, and adds bo.
"""

import numpy as np
import ml_dtypes

import concourse.bass as bass
import concourse.mybir as mybir
import concourse.tile as tile
from concourse import bacc

BF16 = ml_dtypes.bfloat16
F32 = mybir.dt.float32
BF = mybir.dt.bfloat16

B, S, H = 2, 2048, 2048
NH, KVH, D = 16, 4, 128
G = NH // KVH  # q heads per kv head / per core
N_CORES = 8
SCALE = 1.0 / float(np.sqrt(D))

SQ = 512              # q-chunk width
NQC = S // SQ         # 4 q chunks
NKT = S // 128        # 16 kv tiles / token tiles
NHT = H // 128        # 16 hidden k-tiles
ROWS = G + 2          # 6 projection row-blocks: 4 q heads, k, v
EXPF = mybir.ActivationFunctionType.Exp
IDF = mybir.ActivationFunctionType.Identity


def build_nc(num_devices: int = N_CORES) -> bass.Bass:
    nc = bacc.Bacc("TRN2", num_devices=num_devices)

    # packed layouts: [partition, tile, cols] so one DMA moves many tiles
    hTd = nc.dram_tensor("hTd", [128, NHT, S], BF, kind="ExternalInput").ap()
    wqd = nc.dram_tensor("wqd", [128, NHT, ROWS * 128], BF,
                         kind="ExternalInput").ap()
    bqkv = nc.dram_tensor("bqkv", [128, ROWS], F32, kind="ExternalInput").ap()
    cosT = nc.dram_tensor("cosT", [128, S], BF, kind="ExternalInput").ap()
    sinT = nc.dram_tensor("sinT", [128, S], BF, kind="ExternalInput").ap()
    rotT = nc.dram_tensor("rotT", [128, 128], BF, kind="ExternalInput").ap()
    masks2 = nc.dram_tensor("masks2", [128, 256], BF, kind="ExternalInput").ap()
    wod = nc.dram_tensor("wod", [128, G, H], BF, kind="ExternalInput").ap()
    id128 = nc.dram_tensor("id128", [128, 128], BF, kind="ExternalInput").ap()
    out = nc.dram_tensor("out", [S, H], BF, kind="ExternalOutput").ap()

    with tile.TileContext(nc) as tc:
        with (
            tc.tile_pool(name="consts", bufs=1) as consts,
            tc.tile_pool(name="persist", bufs=1) as persist,
            tc.tile_pool(name="hbuf", bufs=2) as hbuf,
            tc.tile_pool(name="work", bufs=3) as work,
            tc.tile_pool(name="work2", bufs=2) as work2,
            tc.tile_pool(name="obuf", bufs=3) as obuf,
            tc.tile_pool(name="psQ", bufs=2, space="PSUM") as psQ,
            tc.tile_pool(name="psS", bufs=2, space="PSUM") as psS,
            tc.tile_pool(name="psAV", bufs=2, space="PSUM") as psAV,
        ):
            # ---- small constants (first in DMA queue) ----
            bias_sb = consts.tile([128, ROWS], F32, tag="bias", name="bias")
            nc.sync.dma_start(out=bias_sb, in_=bqkv)
            rt_sb = consts.tile([128, 128], BF, tag="rt", name="rt")
            nc.sync.dma_start(out=rt_sb, in_=rotT)
            mask_sb = consts.tile([128, 2, 128], BF, tag="mask", name="mask")
            nc.sync.dma_start(out=mask_sb, in_=masks2)
            id_sb = consts.tile([128, 128], BF, tag="id", name="id")
            nc.sync.dma_start(out=id_sb, in_=id128)
            ones_f = consts.tile([128, 1], BF, tag="ones_f", name="ones_f")
            nc.vector.memset(ones_f, 1.0)
            # preload the exp activation table while DMA streams
            warm_in = consts.tile([128, 1], F32, tag="warm_in", name="warm_in")
            nc.vector.memset(warm_in, 0.0)
            warm_out = consts.tile([128, 1], BF, tag="warm_out", name="warm_out")
            nc.scalar.activation(warm_out, warm_in, EXPF)

            # weights + chunk-0 hidden: progressively sized k-tile groups so
            # the first matmuls start after ~330KB while the rest streams
            GRP = [(0, 1), (1, 2), (2, 4), (4, 8), (8, 16)]
            wq_g, h_g = [], []
            h_q = [[None] * 4 for _ in range(NQC)]
            for gi, (lo, hi) in enumerate(GRP):
                wt = persist.tile([128, hi - lo, ROWS * 128], BF,
                                  tag=f"wq{gi}", name=f"wq{gi}")
                nc.sync.dma_start(out=wt, in_=wqd[:, lo:hi, :])
                wq_g.append(wt)
                ht = persist.tile([128, hi - lo, SQ], BF, tag=f"hg{gi}",
                                  name=f"h0_{gi}")
                nc.sync.dma_start(out=ht, in_=hTd[:, lo:hi, 0:SQ])
                h_g.append(ht)
            cos_sb = persist.tile([128, S], BF, tag="cos", name="cos")
            nc.sync.dma_start(out=cos_sb, in_=cosT)
            sin_sb = persist.tile([128, S], BF, tag="sin", name="sin")
            nc.sync.dma_start(out=sin_sb, in_=sinT)
            wo_sb = persist.tile([128, G, H], BF, tag="wo", name="wo")
            nc.sync.dma_start(out=wo_sb, in_=wod)

            def _grp(kt):
                for gi, (lo, hi) in enumerate(GRP):
                    if kt < hi:
                        return gi, kt - lo
                raise AssertionError

            def wq_ap(kt, m):
                gi, o = _grp(kt)
                return wq_g[gi][:, o, m * 128:(m + 1) * 128]

            def h_ap(c, kt):
                if c == 0:
                    gi, o = _grp(kt)
                    return h_g[gi][:, o, :]
                return h_q[c][kt // 4][:, kt % 4, :]

            # persistent activations (bf16)
            qk_sb = [persist.tile([128, S], BF, tag=f"qk{m}", name=f"qk{m}")
                     for m in range(G + 1)]  # 0..3 q heads, 4 = k
            vT_sb = persist.tile([128, S], BF, tag="vT", name="vT")
            v_sb = [persist.tile([128, 128], BF, tag=f"v{j}", name=f"v{j}")
                    for j in range(NKT)]
            xT_sb = [persist.tile([128, S], BF, tag=f"xT{h}", name=f"xT{h}")
                     for h in range(G)]
            kT = qk_sb[G]

            # rows: m 0..3 -> q head m (RoPE), 4 -> k (RoPE), 5 -> v (plain)
            def row_bias(m, ps, cs):
                """PSUM->SBUF copy with bias; returns rope tmp or None."""
                if m == ROWS - 1:
                    nc.scalar.activation(vT_sb[:, cs], ps, IDF,
                                         bias=bias_sb[:, m:m + 1])
                    return None
                tmp = work.tile([128, SQ], BF, tag="tmp", name="tmp")
                nc.scalar.activation(tmp, ps, IDF, bias=bias_sb[:, m:m + 1])
                return tmp

            def row_rope(m, tmp, cs):
                rp = psAV.tile([128, SQ], F32, tag="av", name="rp")
                nc.tensor.matmul(rp, rt_sb, tmp, start=True, stop=True)
                rot = work.tile([128, SQ], BF, tag="rot", name="rot")
                nc.vector.tensor_mul(rot, rp, sin_sb[:, cs])
                tcos = work.tile([128, SQ], BF, tag="tcos", name="tcos")
                nc.vector.tensor_mul(tcos, tmp, cos_sb[:, cs])
                nc.vector.tensor_add(qk_sb[min(m, G)][:, cs], rot, tcos)

            def oproj_fill_ops(c):
                """One micro-op per (token tile, output column block): alloc a
                PSUM tile, 4 accumulated matmuls over heads, copy into the
                per-tile output staging buffer, DMA the row block when done.
                Each closure takes the PSUM pool to use (the one that is free
                during the sweep it is interleaved into)."""
                osbs = {}
                ops = []
                for ti, t in enumerate(range(4 * c, 4 * c + 4)):
                    for n in range(G):
                        def op_fn(pool, t=t, n=n):
                            if t not in osbs:
                                osbs[t] = obuf.tile([128, H], BF, tag="osb",
                                                    name="osb")
                            osb = osbs[t]
                            op = pool.tile([128, SQ], F32,
                                           tag="qkv" if pool is psQ else "av",
                                           name="op")
                            ts_ = slice(t * 128, (t + 1) * 128)
                            for g in range(G):
                                nc.tensor.matmul(
                                    op, xT_sb[g][:, ts_],
                                    wo_sb[:, g, n * SQ:(n + 1) * SQ],
                                    start=(g == 0), stop=(g == G - 1))
                            dst = osb[:, n * SQ:(n + 1) * SQ]
                            if n % 2 == 0:
                                nc.scalar.copy(dst, op)
                            else:
                                nc.vector.tensor_copy(dst, op)
                            if n == G - 1:
                                nc.sync.dma_start(
                                    out=out[t * 128:(t + 1) * 128, :], in_=osb)
                        ops.append(op_fn)
                return ops

            def attn_chunk(c, fill):
                """Returns the hp1 normalization closure for deferred
                emission (behind the next chunk's first QKV row) so the
                denominator matmuls never stall the tensor queue."""
                cs = slice(c * SQ, (c + 1) * SQ)
                njt = 4 * c + 4
                split = njt >= 8  # even/odd denominator chains (j=0,1 full)
                norms = []
                for hp in range(2):
                    h0, h1 = 2 * hp, 2 * hp + 1
                    if hp == 0:
                        av0 = psAV.tile([128, SQ], F32, tag="av", name="av0")
                        av1 = psAV.tile([128, SQ], F32, tag="av", name="av1")
                    else:
                        av0 = psQ.tile([128, SQ], F32, tag="qkv", name="av0b")
                        av1 = psQ.tile([128, SQ], F32, tag="qkv", name="av1b")
                    daccs = [work2.tile([128, 2, SQ], BF, tag=f"dacc{p}",
                                        name=f"dacc{p}")
                             for p in range(2 if split else 1)]
                    pend = None  # (j, ex, off) awaiting its attn@V matmuls
                    for j in range(njt):
                        i = j - 4 * c
                        off = 128 * i if i > 0 else 0
                        sc = psS.tile([128, 2, SQ], F32, tag="sc", name="sc")
                        for hs, h in ((0, h0), (1, h1)):
                            nc.tensor.matmul(
                                sc[:, hs, off:],
                                kT[:, j * 128:(j + 1) * 128],
                                qk_sb[h][:, c * SQ + off:(c + 1) * SQ],
                                start=True, stop=True,
                            )
                        if pend is not None:
                            pj, pex, poff = pend
                            nc.tensor.matmul(av0[:, poff:], v_sb[pj],
                                             pex[:, 0, poff:],
                                             start=(pj == 0), stop=False)
                            nc.tensor.matmul(av1[:, poff:], v_sb[pj],
                                             pex[:, 1, poff:],
                                             start=(pj == 0), stop=False)
                        if j >= 1 and fill:
                            # psQ is free during hp0 (held by hp1's attn@V
                            # accumulators later); psAV frees once hp0's
                            # normalization has read av0/av1
                            fill.pop(0)(psQ if hp == 0 else psAV)
                        ex = work.tile([128, 2, SQ], BF, tag="ex", name="ex")
                        nc.scalar.activation(ex[:, :, off:], sc[:, :, off:],
                                             EXPF, scale=SCALE)
                        if i >= 0:
                            nc.vector.tensor_mul(ex[:, :, off:off + 128],
                                                 ex[:, :, off:off + 128],
                                                 mask_sb)
                        dacc = daccs[j % 2] if split else daccs[0]
                        if j < (2 if split else 1):
                            nc.vector.tensor_copy(dacc, ex)
                        else:
                            nc.vector.tensor_add(dacc[:, :, off:],
                                                 dacc[:, :, off:],
                                                 ex[:, :, off:])
                        pend = (j, ex, off)
                        if hp == 1 and j == 1:
                            norms[0]()  # hp0 norm behind hp1's first matmuls
                    pj, pex, poff = pend
                    nc.tensor.matmul(av0[:, poff:], v_sb[pj], pex[:, 0, poff:],
                                     start=(pj == 0), stop=True)
                    nc.tensor.matmul(av1[:, poff:], v_sb[pj], pex[:, 1, poff:],
                                     start=(pj == 0), stop=True)

                    def norm(hp=hp, av0=av0, av1=av1, daccs=daccs):
                        for hs, av in ((0, av0), (1, av1)):
                            h = 2 * hp + hs
                            dn = psS.tile([1, SQ], F32, tag="sc", name="dn")
                            for p, dacc in enumerate(daccs):
                                nc.tensor.matmul(dn, ones_f, dacc[:, hs, :],
                                                 start=(p == 0),
                                                 stop=(p == len(daccs) - 1))
                            rd = work2.tile([1, SQ], F32, tag="rd", name="rd")
                            nc.vector.reciprocal_approx_fast(rd, dn)
                            rdb = work2.tile([128, SQ], F32, tag="rdb",
                                             name="rdb")
                            nc.gpsimd.partition_broadcast(rdb, rd)
                            nc.vector.tensor_mul(xT_sb[h][:, cs], av, rdb)
                    norms.append(norm)
                return norms[1]

            ROW_ORDER = (G, ROWS - 1, 0, 1, 2, 3)  # k, v, q0..q3
            pending_norm = None
            for c in range(NQC):
                cs = slice(c * SQ, (c + 1) * SQ)
                # prefetch next chunk's hidden tiles
                if c + 1 < NQC:
                    for q in range(4):
                        ht = hbuf.tile([128, 4, SQ], BF, tag=f"h{q}",
                                       name=f"h{c + 1}_{q}")
                        nc.sync.dma_start(
                            out=ht,
                            in_=hTd[:, 4 * q:4 * q + 4,
                                    (c + 1) * SQ:(c + 2) * SQ])
                        h_q[c + 1][q] = ht
                # ---- QKV projection + RoPE ----
                if c == 0:
                    # k-tile-outer so compute starts as DMA streams in;
                    # 6 concurrent accumulators spread over all three pools
                    pools = {0: psQ, 1: psQ, 2: psS, 3: psS, 4: psAV, 5: psAV}
                    tags = {0: "qkv", 1: "qkv", 2: "sc", 3: "sc",
                            4: "av", 5: "av"}
                    accs = {m: pools[m].tile([128, SQ], F32, tag=tags[m],
                                             name=f"acc{m}")
                            for m in range(ROWS)}
                    for kt in range(NHT):
                        for m in range(ROWS):
                            nc.tensor.matmul(
                                accs[m], wq_ap(kt, m), h_ap(0, kt),
                                start=(kt == 0), stop=(kt == NHT - 1),
                            )
                    tmps = {m: row_bias(m, accs[m], cs) for m in ROW_ORDER}
                    for m in ROW_ORDER:
                        if tmps[m] is not None:
                            row_rope(m, tmps[m], cs)
                else:
                    prev = None  # stagger rope behind next row's matmuls
                    for m in ROW_ORDER:
                        # k and v rows accumulate in the scores pool so they
                        # don't wait on the previous chunk's hp1 attention
                        pool, tag = (psS, "sc") if m >= G else (psQ, "qkv")
                        ps = pool.tile([128, SQ], F32, tag=tag, name="mm")
                        for kt in range(NHT):
                            nc.tensor.matmul(
                                ps, wq_ap(kt, m), h_ap(c, kt),
                                start=(kt == 0), stop=(kt == NHT - 1),
                            )
                        if pending_norm is not None:
                            # prev chunk's hp1 norm: its denominator matmuls
                            # land behind this k-row so they never wait
                            pending_norm()
                            pending_norm = None
                        if prev is not None:
                            row_rope(prev[0], prev[1], cs)
                            prev = None
                        tmp = row_bias(m, ps, cs)
                        if tmp is not None:
                            prev = (m, tmp)
                    if prev is not None:
                        row_rope(prev[0], prev[1], cs)
                # ---- transpose this chunk's v tiles ----
                for j in range(4 * c, 4 * c + 4):
                    tp = psS.tile([128, 128], BF, tag="sc", name="tp")
                    nc.tensor.transpose(tp, vT_sb[:, j * 128:(j + 1) * 128],
                                        id_sb)
                    nc.scalar.copy(v_sb[j], tp)
                # ---- attention with oproj(c-1) interleaved ----
                fill = oproj_fill_ops(c - 1) if c > 0 else []
                pending_norm = attn_chunk(c, fill)
                for fn in fill:  # leftovers (c=1 only); psQ is still held
                    fn(psAV)     # by hp1's attn@V until the deferred norm
            pending_norm()
            for fi, fn in enumerate(oproj_fill_ops(NQC - 1)):
                fn(psQ if fi % 2 else psAV)
    nc.compile()
    return nc


def make_in_maps(hidden_states, cos, sin, Wq, bq, Wk, bk, Wv, bv, Wo, bo):
    """Host-side shard/pack. Returns list of 8 input dicts."""
    f32 = np.float32
    cosT = np.ascontiguousarray(np.asarray(cos).T).astype(BF16)
    sinT = np.ascontiguousarray(np.asarray(sin).T).astype(BF16)
    R = np.zeros((128, 128), f32)
    for d in range(64):
        R[d, d + 64] = -1.0
        R[d + 64, d] = 1.0
    rotT = np.ascontiguousarray(R.T).astype(BF16)
    # triangular mask for the diagonal 128-block, duplicated per head-pair
    p = np.arange(128)[:, None]
    q = np.arange(128)[None, :]
    tri = (q >= p).astype(BF16)
    masks2 = np.concatenate([tri, tri], axis=1)  # [128, 256]
    id128 = np.eye(128, dtype=BF16)

    in_maps = []
    for core in range(N_CORES):
        b, k = core // 4, core % 4
        hT = np.ascontiguousarray(np.asarray(hidden_states[b]).T)  # [H, S]
        hTd = np.ascontiguousarray(
            hT.reshape(NHT, 128, S).transpose(1, 0, 2)).astype(BF16)
        wq = Wq[512 * k:512 * (k + 1)]            # [512, H]
        wk = Wk[128 * k:128 * (k + 1)]            # [128, H]
        wv = Wv[128 * k:128 * (k + 1)]
        wqkvT = np.ascontiguousarray(
            np.concatenate([wq, wk, wv], axis=0).T)  # [H, 768]
        wqd = np.ascontiguousarray(
            wqkvT.reshape(NHT, 128, ROWS * 128).transpose(1, 0, 2)
        ).astype(BF16)                             # [128, 16, 768]
        bqkv = np.concatenate(
            [bq[512 * k:512 * (k + 1)], bk[128 * k:128 * (k + 1)],
             bv[128 * k:128 * (k + 1)]]
        ).astype(f32).reshape(ROWS, 128).T.copy()  # [128, ROWS]
        woT = np.ascontiguousarray(Wo[:, 512 * k:512 * (k + 1)].T)  # [512, H]
        wod = np.ascontiguousarray(
            woT.reshape(G, 128, H).transpose(1, 0, 2)).astype(BF16)
        in_maps.append({
            "hTd": hTd, "wqd": wqd, "bqkv": bqkv,
            "cosT": cosT, "sinT": sinT, "masks2": masks2, "rotT": rotT,
            "wod": wod, "id128": id128,
        })
    return in_maps


_NC = None


def kernel(**inputs) -> np.ndarray:
    global _NC
    from concourse.bass_utils import run_bass_kernel_spmd

    if _NC is None:
        _NC = build_nc()
    in_maps = make_in_maps(**inputs)
    res = run_bass_kernel_spmd(_NC, in_maps, core_ids=list(range(N_CORES)))
    out = np.zeros((B, S, H), np.float32)
    for core in range(N_CORES):
        out[core // 4] += np.asarray(res.results[core]["out"], np.float32)
    out += np.asarray(inputs["bo"], np.float32)
    return out
